# revision 9
# baseline (speedup 1.0000x reference)
"""GCN residual block on 8 Trainium2 NeuronCores — v2.

y = relu(gcn_conv(x)) -> relu(@W_lin + b_lin) -> + x

Changes vs v1:
  - Packed bucket streams: slot ranges per (group, bucket) are cumsum of
    cross-core max counts (no per-(g,b) ceil-to-128) -> ~25% fewer gather
    descriptors. Tiles cut every 128 slots; a tile straddling a group
    boundary is matmul'ed once per group with sentinel-deadened rows.
  - Self-loops dropped from the gather; per group one dense bf16 tile of
    the core's own rows (xown16 input) + diagonal sw built from dinv^2.
  - Output kept in transposed [H, NPAD] orientation (residual added from a
    transposed xot input); host transposes back. Saves the PE transpose.
  - 4 SWDGE queues (bucket b -> queue b), gather calls chunked to <= ring
    size and emitted round-robin across buckets so all rings stay fed.
"""

import sys

sys.path.insert(0, "/opt/trn_rl_repo")

import numpy as np
import ml_dtypes
from contextlib import ExitStack

import concourse.bass as bass
import concourse.mybir as mybir
import concourse.tile as tile
from concourse import bacc
from concourse.bass_utils import run_bass_kernel_spmd

N_NODES = 100000
N_EDGES = 1600000
H = 128
NCORES = 8
NPC = N_NODES // NCORES  # 12500
P = 128
NG = (NPC + P - 1) // P  # 98
NPAD = NG * P  # 12544
NBUK = 4
WIN = 25000
SPAN = 12
SENT = 254.0  # sentinel dst slot -> sw row all zero

F32 = mybir.dt.float32
BF16 = mybir.dt.bfloat16
I16 = mybir.dt.int16

TRACE = False
LAST_RESULT = None
LAST_NC = None
LAST_IN_MAPS = None
GATHER_ONLY = False
REPEAT = 1
NQUEUES = 4
CHUNK_TILES = 10**9  # chunking hurts on HW; one call per (span, bucket)
SCRATCH = 49152  # ring 3072 descs: holds a whole queue's calls -> no stalls
GDT = "bf16"  # gather dtype: "bf16" | "fp8" (e3m4, 128B rows via raw inst)


def _dma_gather_narrow(gp, out_ap, in_ap, idxs_ap, num_idxs, elem_size,
                       elem_step, queue_num=0):
    """dma_gather for elem_size_bytes % 128 == 0 (bypasses the %256 assert).

    Non-transpose, DRAM source, no prepare_only. Mirrors
    BassGpSimd.dma_gather's lowering for that subset.
    """
    from concourse.bass import MemorySpace
    from concourse import ap_utils

    gp._assert_queue_num(queue_num)
    assert idxs_ap.dtype == mybir.dt.int16
    assert in_ap.dtype == out_ap.dtype
    dsz = mybir.dt.size(in_ap.dtype)
    assert (elem_size * dsz) % 128 == 0
    assert in_ap.space == MemorySpace.DRAM
    assert idxs_ap.space == MemorySpace.SBUF
    assert out_ap.space == MemorySpace.SBUF
    assert ap_utils.ap_is_contiguous(out_ap.ap[1:])
    assert ap_utils.ap_is_contiguous(idxs_ap.ap[1:])
    assert in_ap.ap[0][0] == elem_step
    stride_bytes = elem_step * dsz
    assert stride_bytes % 256 == 0 and stride_bytes // 256 < 256
    assert in_ap.ap[-1][1] == out_ap.ap[-1][1] == elem_size
    _in_ap = gp.lower_ap_dma(in_ap, for_custom_bir_dma=True)
    _idxs_ap = gp.lower_ap(idxs_ap)
    _out_ap = gp.lower_ap(out_ap)
    return gp.add_instruction(
        mybir.InstDMAGatherAnt(
            name=gp.bass.get_next_instruction_name(),
            ins=[*_in_ap, _idxs_ap, gp.lower_val_access(gp.to_reg(num_idxs))],
            outs=[_out_ap],
            transpose=False,
            num_idxs=num_idxs,
            elem_size=elem_size,
            stride_bytes_256=stride_bytes // 256,
            gen_mode=0,
            single_packet=False,
            queue_num=queue_num,
            sbuf_tokens_per_rank=0,
            sbuf_free_dim_per_rank=0,
            sbuf_free_dim_pad_per_rank=0,
            sbuf_byte_offset=0,
        )
    )


def _preprocess(x, edge_index):
    src = np.ascontiguousarray(edge_index[0]).astype(np.int64)
    dst = np.ascontiguousarray(edge_index[1]).astype(np.int64)
    deg = (np.bincount(dst, minlength=N_NODES) + 1).astype(np.float64)
    dinv = 1.0 / np.sqrt(deg)
    norm = (dinv[src] * dinv[dst]).astype(np.float32)
    loopw = (dinv * dinv).astype(np.float32)

    core = dst // NPC
    d_loc = dst - core * NPC
    g_all = d_loc >> 7
    b_all = src // WIN

    counts = np.zeros((NCORES, NG, NBUK), dtype=np.int64)
    per_core = []
    for c in range(NCORES):
        m = core == c
        s_c, g_c, b_c, w_c, d_c = src[m], g_all[m], b_all[m], norm[m], d_loc[m]
        key = g_c * NBUK + b_c
        order = np.argsort(key, kind="stable")
        s_c, g_c, b_c, w_c, d_c, key = (
            s_c[order], g_c[order], b_c[order], w_c[order], d_c[order],
            key[order],
        )
        counts[c] = np.bincount(key, minlength=NG * NBUK).reshape(NG, NBUK)
        per_core.append((s_c, g_c, b_c, w_c, d_c, key))

    maxcnt = counts.max(axis=0)  # [NG, NBUK]
    # packed slot layout per bucket stream
    S = np.zeros((NG, NBUK), dtype=np.int64)
    S[1:] = np.cumsum(maxcnt[:-1], axis=0)
    L = S[-1] + maxcnt[-1]  # [NBUK] stream lengths
    NT_b = ((L + P - 1) // P).astype(np.int64)  # tiles per bucket

    tg0 = S // P
    tg1 = (S + maxcnt + P - 1) // P
    tg1 = np.maximum(tg1, tg0)  # maxcnt==0 -> empty
    ncols_gb = (tg1 - tg0) * (maxcnt > 0)  # cols per (g, b)

    # meta column ids: walk g -> (b, tiles) -> loop col
    col_base = np.zeros((NG, NBUK + 1), dtype=np.int64)
    nxt = 0
    for g in range(NG):
        for b in range(NBUK):
            col_base[g, b] = nxt
            nxt += int(ncols_gb[g, b])
        col_base[g, NBUK] = nxt
        nxt += 1
    NCOL = nxt

    gidx_all, meta_all = [], []
    for c in range(NCORES):
        s_c, g_c, b_c, w_c, d_c, key = per_core[c]
        cnt = counts[c].reshape(-1)
        starts = np.concatenate([[0], np.cumsum(cnt)[:-1]])
        rank = np.arange(len(s_c)) - starts[key]
        pos = S[g_c, b_c] + rank
        t_e = pos // P
        p_e = pos % P

        idx_wrapped = []
        for b in range(NBUK):
            arr = np.zeros(int(NT_b[b]) * P, dtype=np.int16)
            mb = b_c == b
            arr[pos[mb]] = (s_c[mb] - b * WIN).astype(np.int16)
            idx_wrapped.append(
                np.ascontiguousarray(np.tile(arr.reshape(-1, 16).T, (8, 1)))
            )

        meta = np.zeros((P, NCOL * 2), dtype=np.float32)
        meta[:, 0::2] = SENT
        col_e = col_base[g_c, b_c] + (t_e - tg0[g_c, b_c])
        meta[p_e, 2 * col_e] = (d_c & 127).astype(np.float32)
        meta[p_e, 2 * col_e + 1] = w_c
        # loop cols
        own = np.zeros(NPAD, dtype=np.float32)
        own[:NPC] = loopw[c * NPC : (c + 1) * NPC]
        for g in range(NG):
            cl = col_base[g, NBUK]
            nvalid = min(P, NPC - g * P)
            meta[:nvalid, 2 * cl] = np.arange(nvalid, dtype=np.float32)
            meta[:, 2 * cl + 1] = own[g * P : (g + 1) * P]
        gidx_all.append(idx_wrapped)
        meta_all.append(meta)

    layout = {
        "S": S,
        "maxcnt": maxcnt,
        "NT_b": NT_b,
        "tg0": tg0,
        "tg1": tg1,
        "ncols_gb": ncols_gb,
        "col_base": col_base,
        "NCOL": NCOL,
    }
    return gidx_all, meta_all, layout


def _build_program(layout):
    S = layout["S"]
    maxcnt = layout["maxcnt"]
    NT_b = layout["NT_b"]
    tg0 = layout["tg0"]
    tg1 = layout["tg1"]
    ncols_gb = layout["ncols_gb"]
    col_base = layout["col_base"]
    NCOL = layout["NCOL"]

    nc = bacc.Bacc(
        "TRN2", target_bir_lowering=False, debug=False, num_devices=NCORES,
        num_swdge_queues=NQUEUES, dynamic_dma_scratch_size=SCRATCH,
    )

    F8 = mybir.dt.float8e3
    if GDT == "fp8":
        x8_d = nc.dram_tensor("x8", [N_NODES, 2 * H], F8,
                              kind="ExternalInput")
    else:
        x16_d = nc.dram_tensor("x16", [N_NODES, H], BF16,
                               kind="ExternalInput")
    # [P, NG*H]: column block g holds group g's 128 rows on partitions
    xown16_d = nc.dram_tensor("xown16", [P, NG * H], BF16,
                              kind="ExternalInput")
    idx_d = [
        nc.dram_tensor(f"idx{b}", [P, int(NT_b[b]) * 8], I16,
                       kind="ExternalInput")
        for b in range(NBUK)
    ]
    meta_d = nc.dram_tensor("meta", [P, NCOL * 2], F32, kind="ExternalInput")
    xot_d = nc.dram_tensor("xot", [H, NPAD], F32, kind="ExternalInput")
    iota_d = nc.dram_tensor("iota", [P, P], BF16, kind="ExternalInput")
    wg_d = nc.dram_tensor("wg", [H, H], F32, kind="ExternalInput")
    wl_d = nc.dram_tensor("wl", [H, H], F32, kind="ExternalInput")
    bg_d = nc.dram_tensor("bg", [H, 1], F32, kind="ExternalInput")
    bl_d = nc.dram_tensor("bl", [H, 1], F32, kind="ExternalInput")
    out_d = nc.dram_tensor("out", [H, NPAD], F32, kind="ExternalOutput")

    spans = [(g0, min(g0 + SPAN, NG)) for g0 in range(0, NG, SPAN)]
    # per-span, per-bucket tile ranges
    span_rng = []  # list of [(ts, te)] * NBUK
    for g0, g1 in spans:
        rng = []
        for b in range(NBUK):
            ts = int(tg0[g0, b])
            te = int(min(tg1[g1 - 1, b], NT_b[b]))
            te = max(te, ts)
            rng.append((ts, te))
        span_rng.append(rng)
    max_tiles_b = [
        max(rng[b][1] - rng[b][0] for rng in span_rng) for b in range(NBUK)
    ]

    # per-span meta column ranges
    span_cols = []
    for g0, g1 in spans:
        c_lo = int(col_base[g0, 0])
        c_hi = int(col_base[g1 - 1, NBUK]) + 1
        span_cols.append((c_lo, c_hi))
    max_cols_sp = max(c_hi - c_lo for c_lo, c_hi in span_cols)

    with tile.TileContext(nc) as tc, ExitStack() as ctx:
        constp = ctx.enter_context(tc.tile_pool(name="const", bufs=1))
        gatherp = ctx.enter_context(tc.tile_pool(name="gather", bufs=2))
        idxp = ctx.enter_context(tc.tile_pool(name="idx", bufs=2))
        metap = ctx.enter_context(tc.tile_pool(name="meta", bufs=2))
        loopp = ctx.enter_context(tc.tile_pool(name="loop", bufs=3))
        xotp = ctx.enter_context(tc.tile_pool(name="xot", bufs=2))
        swp = ctx.enter_context(tc.tile_pool(name="sw", bufs=6))
        workp = ctx.enter_context(tc.tile_pool(name="work", bufs=3))
        aggp = ctx.enter_context(tc.tile_pool(name="agg", bufs=2, space="PSUM"))
        chainp = ctx.enter_context(
            tc.tile_pool(name="chain", bufs=2, space="PSUM")
        )

        iota_s = constp.tile([P, P], BF16, tag="iota")
        wg_s = constp.tile([H, H], F32, tag="wg")
        wl_s = constp.tile([H, H], F32, tag="wl")
        bg_s = constp.tile([H, 1], F32, tag="bg")
        bl_s = constp.tile([H, 1], F32, tag="bl")
        for sb, dr in [
            (iota_s, iota_d), (wg_s, wg_d), (wl_s, wl_d),
            (bg_s, bg_d), (bl_s, bl_d),
        ]:
            nc.sync.dma_start(sb[:], dr[:, :])

        for rep in range(REPEAT):
          for si, (g0, g1) in enumerate(spans):
            rng = span_rng[si]
            c_lo, c_hi = span_cols[si]
            gdt = F8 if GDT == "fp8" else BF16
            gbs = [
                gatherp.tile([P, max_tiles_b[b], H], gdt, tag=f"gb{b}",
                             name=f"gb{b}_s{si}")
                for b in range(NBUK)
            ]
            eng = [nc.sync, nc.scalar]
            idx_sp = [
                idxp.tile([P, max_tiles_b[b] * 8], I16, tag=f"idx{b}",
                          name=f"idx{b}_s{si}")
                for b in range(NBUK)
            ]
            for b in range(NBUK):
                ts, te = rng[b]
                eng[b % 2].dma_start(
                    idx_sp[b][:, 0 : (te - ts) * 8],
                    idx_d[b][:, ts * 8 : te * 8],
                )
            meta_sp = metap.tile([P, max_cols_sp * 2], F32, tag="meta",
                                 name=f"meta_s{si}")
            nc.scalar.dma_start(
                meta_sp[:, 0 : 2 * (c_hi - c_lo)],
                meta_d[:, 2 * c_lo : 2 * c_hi],
            )
            # interleaved chunked gathers: all queues fed round-robin
            chunks = [[] for _ in range(NBUK)]
            for b in range(NBUK):
                ts, te = rng[b]
                for t0 in range(ts, te, CHUNK_TILES):
                    tn = min(CHUNK_TILES, te - t0)
                    chunks[b].append((t0 - ts, (t0 - ts) * 8, tn))
            for i in range(max(len(ch) for ch in chunks)):
                for b in range(NBUK):
                    if i >= len(chunks[b]):
                        continue
                    lt, c0, tn = chunks[b][i]
                    ni = tn * P
                    if GDT == "fp8":
                        _dma_gather_narrow(
                            nc.gpsimd,
                            gbs[b][:, lt : lt + tn, :],
                            x8_d[b * WIN : (b + 1) * WIN, 0:H],
                            idx_sp[b][:, c0 : c0 + tn * 8],
                            ni,
                            H,
                            2 * H,
                            queue_num=b % NQUEUES,
                        )
                    else:
                        nc.gpsimd.dma_gather(
                            gbs[b][:, lt : lt + tn, :],
                            x16_d[b * WIN : (b + 1) * WIN, :],
                            idx_sp[b][:, c0 : c0 + tn * 8],
                            ni,
                            ni,
                            H,
                            single_packet=False,
                            queue_num=b % NQUEUES,
                        )
            xot_sp = xotp.tile([H, (g1 - g0) * P], F32, tag="xot")
            nc.scalar.dma_start(xot_sp[:], xot_d[:, g0 * P : g1 * P])
            xo16_sp = loopp.tile([P, (g1 - g0) * H], BF16, tag="xo16")
            nc.sync.dma_start(xo16_sp[:], xown16_d[:, g0 * H : g1 * H])

            for g in range(g0, g1):
                if GATHER_ONLY:
                    continue
                agg_ps = aggp.tile([H, P], F32, tag="agg")
                ntg = int(ncols_gb[g].sum()) + 1
                ti = 0
                for b in range(NBUK):
                    if ncols_gb[g, b] == 0:
                        continue
                    ts_b = rng[b][0]
                    for t in range(int(tg0[g, b]), int(tg1[g, b])):
                        col = int(col_base[g, b]) + t - int(tg0[g, b])
                        rc = col - c_lo
                        sw = swp.tile([P, P], BF16, tag="sw")
                        nc.vector.tensor_scalar(
                            sw[:],
                            iota_s[:],
                            meta_sp[:, 2 * rc : 2 * rc + 1],
                            meta_sp[:, 2 * rc + 1 : 2 * rc + 2],
                            op0=mybir.AluOpType.is_equal,
                            op1=mybir.AluOpType.mult,
                        )
                        nc.tensor.matmul(
                            agg_ps[:],
                            lhsT=gbs[b][:, t - ts_b, :],
                            rhs=sw[:],
                            start=(ti == 0),
                            stop=False,
                        )
                        ti += 1
                # self-loop dense tile
                cl = int(col_base[g, NBUK]) - c_lo
                sw = swp.tile([P, P], BF16, tag="sw")
                nc.vector.tensor_scalar(
                    sw[:],
                    iota_s[:],
                    meta_sp[:, 2 * cl : 2 * cl + 1],
                    meta_sp[:, 2 * cl + 1 : 2 * cl + 2],
                    op0=mybir.AluOpType.is_equal,
                    op1=mybir.AluOpType.mult,
                )
                nc.tensor.matmul(
                    agg_ps[:],
                    lhsT=xo16_sp[:, (g - g0) * H : (g - g0 + 1) * H],
                    rhs=sw[:],
                    start=(ti == 0), stop=True,
                )
                # fp32 chain in transposed orientation [h x d]
                aggT = workp.tile([H, P], F32, tag="aggT")
                nc.scalar.copy(aggT[:], agg_ps[:])
                h1_ps = chainp.tile([H, P], F32, tag="h1ps")
                nc.tensor.matmul(h1_ps[:], lhsT=wg_s[:], rhs=aggT[:],
                                 start=True, stop=True)
                h1 = workp.tile([H, P], F32, tag="h1")
                nc.scalar.activation(
                    h1[:], h1_ps[:], mybir.ActivationFunctionType.Relu,
                    bias=bg_s[:, 0:1], scale=1.0,
                )
                h2_ps = chainp.tile([H, P], F32, tag="h2ps")
                nc.tensor.matmul(h2_ps[:], lhsT=wl_s[:], rhs=h1[:],
                                 start=True, stop=True)
                outt = workp.tile([H, P], F32, tag="outt")
                nc.scalar.activation(
                    outt[:], h2_ps[:], mybir.ActivationFunctionType.Relu,
                    bias=bl_s[:, 0:1], scale=1.0,
                )
                res = workp.tile([H, P], F32, tag="res")
                nc.vector.tensor_tensor(
                    out=res[:],
                    in0=outt[:],
                    in1=xot_sp[:, (g - g0) * P : (g - g0 + 1) * P],
                    op=mybir.AluOpType.add,
                )
                nc.sync.dma_start(out_d[:, g * P : (g + 1) * P], res[:])

    nc.compile()
    return nc


def kernel(x, edge_index, W_gcn, b_gcn, W_lin, b_lin):
    x = np.asarray(x, dtype=np.float32)
    edge_index = np.asarray(edge_index)
    W_gcn = np.asarray(W_gcn, dtype=np.float32)
    b_gcn = np.asarray(b_gcn, dtype=np.float32)
    W_lin = np.asarray(W_lin, dtype=np.float32)
    b_lin = np.asarray(b_lin, dtype=np.float32)

    gidx_all, meta_all, layout = _preprocess(x, edge_index)
    nc = _build_program(layout)

    x16 = x.astype(ml_dtypes.bfloat16)
    x8 = None
    if GDT == "fp8":
        x8 = np.zeros((N_NODES, 2 * H), dtype=ml_dtypes.float8_e3m4)
        x8[:, :H] = x.astype(ml_dtypes.float8_e3m4)
    iota = np.tile(np.arange(P, dtype=np.float32), (P, 1)).astype(
        ml_dtypes.bfloat16
    )
    bg = b_gcn.reshape(H, 1)
    bl = b_lin.reshape(H, 1)

    in_maps = []
    for c in range(NCORES):
        xpad = np.zeros((NPAD, H), dtype=np.float32)
        xpad[:NPC] = x[c * NPC : (c + 1) * NPC]
        xo16_l = np.ascontiguousarray(
            xpad.astype(ml_dtypes.bfloat16)
            .reshape(NG, P, H).transpose(1, 0, 2).reshape(P, NG * H)
        )
        m = {
            "xown16": xo16_l,
            "meta": meta_all[c],
            "xot": np.ascontiguousarray(xpad.T),
            "iota": iota,
            "wg": W_gcn,
            "wl": W_lin,
            "bg": bg,
            "bl": bl,
        }
        if GDT == "fp8":
            m["x8"] = x8
        else:
            m["x16"] = x16
        for b in range(NBUK):
            m[f"idx{b}"] = gidx_all[c][b]
        in_maps.append(m)

    global LAST_RESULT, LAST_NC, LAST_IN_MAPS
    LAST_NC = nc
    LAST_IN_MAPS = in_maps
    res = run_bass_kernel_spmd(
        nc, in_maps, core_ids=list(range(NCORES)), trace=TRACE
    )
    LAST_RESULT = res
    outs = [res.results[c]["out"].T[:NPC] for c in range(NCORES)]
    return np.concatenate(outs, axis=0).astype(np.float32)


if __name__ == "__main__":
    rng = np.random.default_rng(0)
    x = rng.standard_normal((N_NODES, H), dtype=np.float32)
    ei = rng.integers(0, N_NODES, size=(2, N_EDGES)).astype(np.int32)
    s = 1.0 / np.sqrt(H)
    W1 = rng.uniform(-s, s, (H, H)).astype(np.float32)
    b1 = rng.uniform(-s, s, H).astype(np.float32)
    W2 = rng.uniform(-s, s, (H, H)).astype(np.float32)
    b2 = rng.uniform(-s, s, H).astype(np.float32)
    out = kernel(x=x, edge_index=ei, W_gcn=W1, b_gcn=b1, W_lin=W2, b_lin=b2)
    print(out.shape, out.dtype)


# revision 14
# speedup vs baseline: 1.0484x; 1.0484x over previous
"""GCN residual block on 8 Trainium2 NeuronCores — v2.

y = relu(gcn_conv(x)) -> relu(@W_lin + b_lin) -> + x

Changes vs v1:
  - Packed bucket streams: slot ranges per (group, bucket) are cumsum of
    cross-core max counts (no per-(g,b) ceil-to-128) -> ~25% fewer gather
    descriptors. Tiles cut every 128 slots; a tile straddling a group
    boundary is matmul'ed once per group with sentinel-deadened rows.
  - Self-loops dropped from the gather; per group one dense bf16 tile of
    the core's own rows (xown16 input) + diagonal sw built from dinv^2.
  - Output kept in transposed [H, NPAD] orientation (residual added from a
    transposed xot input); host transposes back. Saves the PE transpose.
  - 4 SWDGE queues (bucket b -> queue b), gather calls chunked to <= ring
    size and emitted round-robin across buckets so all rings stay fed.
"""

import sys

sys.path.insert(0, "/opt/trn_rl_repo")

import numpy as np
import ml_dtypes
from contextlib import ExitStack

import concourse.bass as bass
import concourse.mybir as mybir
import concourse.tile as tile
from concourse import bacc
from concourse.bass_utils import run_bass_kernel_spmd

N_NODES = 100000
N_EDGES = 1600000
H = 128
NCORES = 8
NPC = N_NODES // NCORES  # 12500
P = 128
NG = (NPC + P - 1) // P  # 98
NPAD = NG * P  # 12544
NBUK = 4
WIN = 25000
SPAN = 12
SENT = 254.0  # sentinel dst slot -> sw row all zero

F32 = mybir.dt.float32
BF16 = mybir.dt.bfloat16
I16 = mybir.dt.int16

TRACE = False
LAST_RESULT = None
LAST_NC = None
LAST_IN_MAPS = None
GATHER_ONLY = False
REPEAT = 1
NQUEUES = 4
CHUNK_TILES = 10**9  # chunking hurts on HW; one call per (span, bucket)
SCRATCH = 49152  # ring 3072 descs: holds a whole queue's calls -> no stalls
GDT = "bf16"  # gather dtype: "bf16" | "fp8" (e3m4, 128B rows via raw inst)
OUT_BATCH = False  # one output DMA per span instead of per group
SW_BUFS = 6
AGG_BUFS = 2
CHAIN_BUFS = 3  # 2+6=8 PSUM banks exactly; deeper chain pipelining (-80us)


def _dma_gather_narrow(gp, out_ap, in_ap, idxs_ap, num_idxs, elem_size,
                       elem_step, queue_num=0):
    """dma_gather for elem_size_bytes % 128 == 0 (bypasses the %256 assert).

    Non-transpose, DRAM source, no prepare_only. Mirrors
    BassGpSimd.dma_gather's lowering for that subset.
    """
    from concourse.bass import MemorySpace
    from concourse import ap_utils

    gp._assert_queue_num(queue_num)
    assert idxs_ap.dtype == mybir.dt.int16
    assert in_ap.dtype == out_ap.dtype
    dsz = mybir.dt.size(in_ap.dtype)
    assert (elem_size * dsz) % 128 == 0
    assert in_ap.space == MemorySpace.DRAM
    assert idxs_ap.space == MemorySpace.SBUF
    assert out_ap.space == MemorySpace.SBUF
    assert ap_utils.ap_is_contiguous(out_ap.ap[1:])
    assert ap_utils.ap_is_contiguous(idxs_ap.ap[1:])
    assert in_ap.ap[0][0] == elem_step
    stride_bytes = elem_step * dsz
    assert stride_bytes % 256 == 0 and stride_bytes // 256 < 256
    assert in_ap.ap[-1][1] == out_ap.ap[-1][1] == elem_size
    _in_ap = gp.lower_ap_dma(in_ap, for_custom_bir_dma=True)
    _idxs_ap = gp.lower_ap(idxs_ap)
    _out_ap = gp.lower_ap(out_ap)
    return gp.add_instruction(
        mybir.InstDMAGatherAnt(
            name=gp.bass.get_next_instruction_name(),
            ins=[*_in_ap, _idxs_ap, gp.lower_val_access(gp.to_reg(num_idxs))],
            outs=[_out_ap],
            transpose=False,
            num_idxs=num_idxs,
            elem_size=elem_size,
            stride_bytes_256=stride_bytes // 256,
            gen_mode=0,
            single_packet=False,
            queue_num=queue_num,
            sbuf_tokens_per_rank=0,
            sbuf_free_dim_per_rank=0,
            sbuf_free_dim_pad_per_rank=0,
            sbuf_byte_offset=0,
        )
    )


def _preprocess(x, edge_index):
    src = np.ascontiguousarray(edge_index[0]).astype(np.int64)
    dst = np.ascontiguousarray(edge_index[1]).astype(np.int64)
    deg = (np.bincount(dst, minlength=N_NODES) + 1).astype(np.float64)
    dinv = 1.0 / np.sqrt(deg)
    norm = (dinv[src] * dinv[dst]).astype(np.float32)
    loopw = (dinv * dinv).astype(np.float32)

    core = dst // NPC
    d_loc = dst - core * NPC
    g_all = d_loc >> 7
    b_all = src // WIN

    counts = np.zeros((NCORES, NG, NBUK), dtype=np.int64)
    per_core = []
    for c in range(NCORES):
        m = core == c
        s_c, g_c, b_c, w_c, d_c = src[m], g_all[m], b_all[m], norm[m], d_loc[m]
        key = g_c * NBUK + b_c
        order = np.argsort(key, kind="stable")
        s_c, g_c, b_c, w_c, d_c, key = (
            s_c[order], g_c[order], b_c[order], w_c[order], d_c[order],
            key[order],
        )
        counts[c] = np.bincount(key, minlength=NG * NBUK).reshape(NG, NBUK)
        per_core.append((s_c, g_c, b_c, w_c, d_c, key))

    maxcnt = counts.max(axis=0)  # [NG, NBUK]
    # packed slot layout per bucket stream
    S = np.zeros((NG, NBUK), dtype=np.int64)
    S[1:] = np.cumsum(maxcnt[:-1], axis=0)
    L = S[-1] + maxcnt[-1]  # [NBUK] stream lengths
    NT_b = ((L + P - 1) // P).astype(np.int64)  # tiles per bucket

    tg0 = S // P
    tg1 = (S + maxcnt + P - 1) // P
    tg1 = np.maximum(tg1, tg0)  # maxcnt==0 -> empty
    ncols_gb = (tg1 - tg0) * (maxcnt > 0)  # cols per (g, b)

    # meta column ids: walk g -> (b, tiles) -> loop col
    col_base = np.zeros((NG, NBUK + 1), dtype=np.int64)
    nxt = 0
    for g in range(NG):
        for b in range(NBUK):
            col_base[g, b] = nxt
            nxt += int(ncols_gb[g, b])
        col_base[g, NBUK] = nxt
        nxt += 1
    NCOL = nxt

    gidx_all, meta_all = [], []
    for c in range(NCORES):
        s_c, g_c, b_c, w_c, d_c, key = per_core[c]
        cnt = counts[c].reshape(-1)
        starts = np.concatenate([[0], np.cumsum(cnt)[:-1]])
        rank = np.arange(len(s_c)) - starts[key]
        pos = S[g_c, b_c] + rank
        t_e = pos // P
        p_e = pos % P

        idx_wrapped = []
        for b in range(NBUK):
            arr = np.zeros(int(NT_b[b]) * P, dtype=np.int16)
            mb = b_c == b
            arr[pos[mb]] = (s_c[mb] - b * WIN).astype(np.int16)
            idx_wrapped.append(
                np.ascontiguousarray(np.tile(arr.reshape(-1, 16).T, (8, 1)))
            )

        meta = np.zeros((P, NCOL * 2), dtype=np.float32)
        meta[:, 0::2] = SENT
        col_e = col_base[g_c, b_c] + (t_e - tg0[g_c, b_c])
        meta[p_e, 2 * col_e] = (d_c & 127).astype(np.float32)
        meta[p_e, 2 * col_e + 1] = w_c
        # loop cols
        own = np.zeros(NPAD, dtype=np.float32)
        own[:NPC] = loopw[c * NPC : (c + 1) * NPC]
        for g in range(NG):
            cl = col_base[g, NBUK]
            nvalid = min(P, NPC - g * P)
            meta[:nvalid, 2 * cl] = np.arange(nvalid, dtype=np.float32)
            meta[:, 2 * cl + 1] = own[g * P : (g + 1) * P]
        gidx_all.append(idx_wrapped)
        meta_all.append(meta)

    layout = {
        "S": S,
        "maxcnt": maxcnt,
        "NT_b": NT_b,
        "tg0": tg0,
        "tg1": tg1,
        "ncols_gb": ncols_gb,
        "col_base": col_base,
        "NCOL": NCOL,
    }
    return gidx_all, meta_all, layout


def _build_program(layout):
    S = layout["S"]
    maxcnt = layout["maxcnt"]
    NT_b = layout["NT_b"]
    tg0 = layout["tg0"]
    tg1 = layout["tg1"]
    ncols_gb = layout["ncols_gb"]
    col_base = layout["col_base"]
    NCOL = layout["NCOL"]

    nc = bacc.Bacc(
        "TRN2", target_bir_lowering=False, debug=False, num_devices=NCORES,
        num_swdge_queues=NQUEUES, dynamic_dma_scratch_size=SCRATCH,
    )

    F8 = mybir.dt.float8e3
    if GDT == "fp8":
        x8_d = nc.dram_tensor("x8", [N_NODES, 2 * H], F8,
                              kind="ExternalInput")
    else:
        x16_d = nc.dram_tensor("x16", [N_NODES, H], BF16,
                               kind="ExternalInput")
    # [P, NG*H]: column block g holds group g's 128 rows on partitions
    xown16_d = nc.dram_tensor("xown16", [P, NG * H], BF16,
                              kind="ExternalInput")
    idx_d = [
        nc.dram_tensor(f"idx{b}", [P, int(NT_b[b]) * 8], I16,
                       kind="ExternalInput")
        for b in range(NBUK)
    ]
    meta_d = nc.dram_tensor("meta", [P, NCOL * 2], F32, kind="ExternalInput")
    xot_d = nc.dram_tensor("xot", [H, NPAD], F32, kind="ExternalInput")
    iota_d = nc.dram_tensor("iota", [P, P], BF16, kind="ExternalInput")
    wg_d = nc.dram_tensor("wg", [H, H], F32, kind="ExternalInput")
    wl_d = nc.dram_tensor("wl", [H, H], F32, kind="ExternalInput")
    bg_d = nc.dram_tensor("bg", [H, 1], F32, kind="ExternalInput")
    bl_d = nc.dram_tensor("bl", [H, 1], F32, kind="ExternalInput")
    out_d = nc.dram_tensor("out", [H, NPAD], F32, kind="ExternalOutput")

    spans = [(g0, min(g0 + SPAN, NG)) for g0 in range(0, NG, SPAN)]
    # per-span, per-bucket tile ranges
    span_rng = []  # list of [(ts, te)] * NBUK
    for g0, g1 in spans:
        rng = []
        for b in range(NBUK):
            ts = int(tg0[g0, b])
            te = int(min(tg1[g1 - 1, b], NT_b[b]))
            te = max(te, ts)
            rng.append((ts, te))
        span_rng.append(rng)
    max_tiles_b = [
        max(rng[b][1] - rng[b][0] for rng in span_rng) for b in range(NBUK)
    ]

    # per-span meta column ranges
    span_cols = []
    for g0, g1 in spans:
        c_lo = int(col_base[g0, 0])
        c_hi = int(col_base[g1 - 1, NBUK]) + 1
        span_cols.append((c_lo, c_hi))
    max_cols_sp = max(c_hi - c_lo for c_lo, c_hi in span_cols)

    with tile.TileContext(nc) as tc, ExitStack() as ctx:
        constp = ctx.enter_context(tc.tile_pool(name="const", bufs=1))
        gatherp = ctx.enter_context(tc.tile_pool(name="gather", bufs=2))
        idxp = ctx.enter_context(tc.tile_pool(name="idx", bufs=2))
        metap = ctx.enter_context(tc.tile_pool(name="meta", bufs=2))
        loopp = ctx.enter_context(tc.tile_pool(name="loop", bufs=3))
        xotp = ctx.enter_context(tc.tile_pool(name="xot", bufs=2))
        swp = ctx.enter_context(tc.tile_pool(name="sw", bufs=SW_BUFS))
        workp = ctx.enter_context(tc.tile_pool(name="work", bufs=3))
        outp = ctx.enter_context(tc.tile_pool(name="outsp", bufs=2))
        aggp = ctx.enter_context(
            tc.tile_pool(name="agg", bufs=AGG_BUFS, space="PSUM")
        )
        chainp = ctx.enter_context(
            tc.tile_pool(name="chain", bufs=CHAIN_BUFS, space="PSUM")
        )

        iota_s = constp.tile([P, P], BF16, tag="iota")
        wg_s = constp.tile([H, H], F32, tag="wg")
        wl_s = constp.tile([H, H], F32, tag="wl")
        bg_s = constp.tile([H, 1], F32, tag="bg")
        bl_s = constp.tile([H, 1], F32, tag="bl")
        for sb, dr in [
            (iota_s, iota_d), (wg_s, wg_d), (wl_s, wl_d),
            (bg_s, bg_d), (bl_s, bl_d),
        ]:
            nc.sync.dma_start(sb[:], dr[:, :])

        for rep in range(REPEAT):
          for si, (g0, g1) in enumerate(spans):
            rng = span_rng[si]
            c_lo, c_hi = span_cols[si]
            gdt = F8 if GDT == "fp8" else BF16
            gbs = [
                gatherp.tile([P, max_tiles_b[b], H], gdt, tag=f"gb{b}",
                             name=f"gb{b}_s{si}")
                for b in range(NBUK)
            ]
            eng = [nc.sync, nc.scalar]
            idx_sp = [
                idxp.tile([P, max_tiles_b[b] * 8], I16, tag=f"idx{b}",
                          name=f"idx{b}_s{si}")
                for b in range(NBUK)
            ]
            for b in range(NBUK):
                ts, te = rng[b]
                eng[b % 2].dma_start(
                    idx_sp[b][:, 0 : (te - ts) * 8],
                    idx_d[b][:, ts * 8 : te * 8],
                )
            meta_sp = metap.tile([P, max_cols_sp * 2], F32, tag="meta",
                                 name=f"meta_s{si}")
            nc.scalar.dma_start(
                meta_sp[:, 0 : 2 * (c_hi - c_lo)],
                meta_d[:, 2 * c_lo : 2 * c_hi],
            )
            # interleaved chunked gathers: all queues fed round-robin
            chunks = [[] for _ in range(NBUK)]
            for b in range(NBUK):
                ts, te = rng[b]
                for t0 in range(ts, te, CHUNK_TILES):
                    tn = min(CHUNK_TILES, te - t0)
                    chunks[b].append((t0 - ts, (t0 - ts) * 8, tn))
            for i in range(max(len(ch) for ch in chunks)):
                for b in range(NBUK):
                    if i >= len(chunks[b]):
                        continue
                    lt, c0, tn = chunks[b][i]
                    ni = tn * P
                    if GDT == "fp8":
                        _dma_gather_narrow(
                            nc.gpsimd,
                            gbs[b][:, lt : lt + tn, :],
                            x8_d[b * WIN : (b + 1) * WIN, 0:H],
                            idx_sp[b][:, c0 : c0 + tn * 8],
                            ni,
                            H,
                            2 * H,
                            queue_num=b % NQUEUES,
                        )
                    else:
                        nc.gpsimd.dma_gather(
                            gbs[b][:, lt : lt + tn, :],
                            x16_d[b * WIN : (b + 1) * WIN, :],
                            idx_sp[b][:, c0 : c0 + tn * 8],
                            ni,
                            ni,
                            H,
                            single_packet=False,
                            queue_num=b % NQUEUES,
                        )
            xot_sp = xotp.tile([H, (g1 - g0) * P], F32, tag="xot")
            nc.scalar.dma_start(xot_sp[:], xot_d[:, g0 * P : g1 * P])
            xo16_sp = loopp.tile([P, (g1 - g0) * H], BF16, tag="xo16")
            nc.sync.dma_start(xo16_sp[:], xown16_d[:, g0 * H : g1 * H])
            if OUT_BATCH and not GATHER_ONLY:
                outsp = outp.tile([H, (g1 - g0) * P], F32, tag="res",
                                  name=f"outsp_{si}")

            for g in range(g0, g1):
                if GATHER_ONLY:
                    continue
                agg_ps = aggp.tile([H, P], F32, tag="agg")
                ntg = int(ncols_gb[g].sum()) + 1
                ti = 0
                for b in range(NBUK):
                    if ncols_gb[g, b] == 0:
                        continue
                    ts_b = rng[b][0]
                    for t in range(int(tg0[g, b]), int(tg1[g, b])):
                        col = int(col_base[g, b]) + t - int(tg0[g, b])
                        rc = col - c_lo
                        sw = swp.tile([P, P], BF16, tag="sw")
                        nc.vector.tensor_scalar(
                            sw[:],
                            iota_s[:],
                            meta_sp[:, 2 * rc : 2 * rc + 1],
                            meta_sp[:, 2 * rc + 1 : 2 * rc + 2],
                            op0=mybir.AluOpType.is_equal,
                            op1=mybir.AluOpType.mult,
                        )
                        nc.tensor.matmul(
                            agg_ps[:],
                            lhsT=gbs[b][:, t - ts_b, :],
                            rhs=sw[:],
                            start=(ti == 0),
                            stop=False,
                        )
                        ti += 1
                # self-loop dense tile
                cl = int(col_base[g, NBUK]) - c_lo
                sw = swp.tile([P, P], BF16, tag="sw")
                nc.vector.tensor_scalar(
                    sw[:],
                    iota_s[:],
                    meta_sp[:, 2 * cl : 2 * cl + 1],
                    meta_sp[:, 2 * cl + 1 : 2 * cl + 2],
                    op0=mybir.AluOpType.is_equal,
                    op1=mybir.AluOpType.mult,
                )
                nc.tensor.matmul(
                    agg_ps[:],
                    lhsT=xo16_sp[:, (g - g0) * H : (g - g0 + 1) * H],
                    rhs=sw[:],
                    start=(ti == 0), stop=True,
                )
                # fp32 chain in transposed orientation [h x d]
                aggT = workp.tile([H, P], F32, tag="aggT")
                nc.scalar.copy(aggT[:], agg_ps[:])
                h1_ps = chainp.tile([H, P], F32, tag="h1ps")
                nc.tensor.matmul(h1_ps[:], lhsT=wg_s[:], rhs=aggT[:],
                                 start=True, stop=True)
                h1 = workp.tile([H, P], F32, tag="h1")
                nc.scalar.activation(
                    h1[:], h1_ps[:], mybir.ActivationFunctionType.Relu,
                    bias=bg_s[:, 0:1], scale=1.0,
                )
                h2_ps = chainp.tile([H, P], F32, tag="h2ps")
                nc.tensor.matmul(h2_ps[:], lhsT=wl_s[:], rhs=h1[:],
                                 start=True, stop=True)
                outt = workp.tile([H, P], F32, tag="outt")
                nc.scalar.activation(
                    outt[:], h2_ps[:], mybir.ActivationFunctionType.Relu,
                    bias=bl_s[:, 0:1], scale=1.0,
                )
                if OUT_BATCH:
                    nc.vector.tensor_tensor(
                        out=outsp[:, (g - g0) * P : (g - g0 + 1) * P],
                        in0=outt[:],
                        in1=xot_sp[:, (g - g0) * P : (g - g0 + 1) * P],
                        op=mybir.AluOpType.add,
                    )
                else:
                    res = workp.tile([H, P], F32, tag="res")
                    nc.vector.tensor_tensor(
                        out=res[:],
                        in0=outt[:],
                        in1=xot_sp[:, (g - g0) * P : (g - g0 + 1) * P],
                        op=mybir.AluOpType.add,
                    )
                    nc.sync.dma_start(out_d[:, g * P : (g + 1) * P], res[:])
            if OUT_BATCH and not GATHER_ONLY:
                nc.sync.dma_start(out_d[:, g0 * P : g1 * P], outsp[:])

    nc.compile()
    return nc


def kernel(x, edge_index, W_gcn, b_gcn, W_lin, b_lin):
    x = np.asarray(x, dtype=np.float32)
    edge_index = np.asarray(edge_index)
    W_gcn = np.asarray(W_gcn, dtype=np.float32)
    b_gcn = np.asarray(b_gcn, dtype=np.float32)
    W_lin = np.asarray(W_lin, dtype=np.float32)
    b_lin = np.asarray(b_lin, dtype=np.float32)

    gidx_all, meta_all, layout = _preprocess(x, edge_index)
    nc = _build_program(layout)

    x16 = x.astype(ml_dtypes.bfloat16)
    x8 = None
    if GDT == "fp8":
        x8 = np.zeros((N_NODES, 2 * H), dtype=ml_dtypes.float8_e3m4)
        x8[:, :H] = x.astype(ml_dtypes.float8_e3m4)
    iota = np.tile(np.arange(P, dtype=np.float32), (P, 1)).astype(
        ml_dtypes.bfloat16
    )
    bg = b_gcn.reshape(H, 1)
    bl = b_lin.reshape(H, 1)

    in_maps = []
    for c in range(NCORES):
        xpad = np.zeros((NPAD, H), dtype=np.float32)
        xpad[:NPC] = x[c * NPC : (c + 1) * NPC]
        xo16_l = np.ascontiguousarray(
            xpad.astype(ml_dtypes.bfloat16)
            .reshape(NG, P, H).transpose(1, 0, 2).reshape(P, NG * H)
        )
        m = {
            "xown16": xo16_l,
            "meta": meta_all[c],
            "xot": np.ascontiguousarray(xpad.T),
            "iota": iota,
            "wg": W_gcn,
            "wl": W_lin,
            "bg": bg,
            "bl": bl,
        }
        if GDT == "fp8":
            m["x8"] = x8
        else:
            m["x16"] = x16
        for b in range(NBUK):
            m[f"idx{b}"] = gidx_all[c][b]
        in_maps.append(m)

    global LAST_RESULT, LAST_NC, LAST_IN_MAPS
    LAST_NC = nc
    LAST_IN_MAPS = in_maps
    res = run_bass_kernel_spmd(
        nc, in_maps, core_ids=list(range(NCORES)), trace=TRACE
    )
    LAST_RESULT = res
    outs = [res.results[c]["out"].T[:NPC] for c in range(NCORES)]
    return np.concatenate(outs, axis=0).astype(np.float32)


if __name__ == "__main__":
    rng = np.random.default_rng(0)
    x = rng.standard_normal((N_NODES, H), dtype=np.float32)
    ei = rng.integers(0, N_NODES, size=(2, N_EDGES)).astype(np.int32)
    s = 1.0 / np.sqrt(H)
    W1 = rng.uniform(-s, s, (H, H)).astype(np.float32)
    b1 = rng.uniform(-s, s, H).astype(np.float32)
    W2 = rng.uniform(-s, s, (H, H)).astype(np.float32)
    b2 = rng.uniform(-s, s, H).astype(np.float32)
    out = kernel(x=x, edge_index=ei, W_gcn=W1, b_gcn=b1, W_lin=W2, b_lin=b2)
    print(out.shape, out.dtype)
